# revision 10
# baseline (speedup 1.0000x reference)
"""AffineEdgeAttention Trainium2 kernel.

out[b, i, j] = head[b, i] . w_h + dep[b, j] . w_d + edge_b
with w_h = edge_W[0, :D], w_d = edge_W[0, D:].

Sharding: data-parallel over batch; 16 batches / 8 cores = 2 per core.

Per core, per batch (memory-bound; ~20.75 MiB of HBM traffic/core):
  - dep[b] loaded first as 4 chunk-pair tiles [128, 2, 768] (contiguous
    384KB row-blocks); as each lands: elementwise *w_d (DVE/GpSimd) then
    free-axis reduce (ACT) -> s_d column chunks [128, 1].
  - s_d [128, 8] + edge_b -> row [1, 1024] via tiny SBUF->SBUF DMA
    (interleaved AP), broadcast to all 128 partitions via K=1 ones-matmul
    into PSUM (PE), plus one PSUM->SBUF copy so DVE adds run in 2x mode.
  - head[b] likewise -> s_h[:, c]; each output chunk [128, 1024] is one
    broadcast-add (s_d_bcast + s_h[:, c]) split across ACT/DVE, written
    into [128, 2, 1024] pair tiles, DMA'd out as 1 MiB stores.
  - input loads issue on the sync HWDGE ring; stores + scatter on the
    scalar HWDGE ring so the two streams don't serialize behind each other.
"""

import sys

import numpy as np

for _p in ("/opt/trn_rl_repo", "/root/.axon_site/_ro/trn_rl_repo"):
    if _p not in sys.path:
        sys.path.insert(0, _p)

import concourse.bacc as bacc
import concourse.bass as bass
import concourse.tile as tile
from concourse import mybir
from concourse.bass_utils import run_bass_kernel_spmd

B, S, D = 16, 1024, 768
N_CORES = 8
BPC = B // N_CORES  # batches per core
P = 128
C = S // P  # 8 row-chunks of 128
NPAIR = C // 2  # 4 chunk-pair tiles per tensor per batch

F32 = mybir.dt.float32

# engine assignment per chunk index 0..7 ("V"=vector, "A"=scalar, "G"=gpsimd)
DEP_MULT_ENG = ["G", "G", "V", "V", "V", "V", "V", "V"]
HEAD_MULT_ENG = ["G", "G", "G", "V", "V", "V", "V", "V"]
HEAD_RED_ENG = ["V", "A", "V", "A", "V", "A", "V", "A"]
OUT_ENG = ["V", "A", "V", "A", "V", "A", "V", "A"]


def build_program() -> bass.Bass:
    nc = bacc.Bacc("TRN2", target_bir_lowering=False, debug=False)
    head = nc.dram_tensor("head", [BPC, S, D], F32, kind="ExternalInput").ap()
    dep = nc.dram_tensor("dep", [BPC, S, D], F32, kind="ExternalInput").ap()
    w = nc.dram_tensor("edge_W", [1, 2 * D], F32, kind="ExternalInput").ap()
    b = nc.dram_tensor("edge_b", [1], F32, kind="ExternalInput").ap()
    out = nc.dram_tensor("out", [BPC, S, S], F32, kind="ExternalOutput").ap()

    # [b, t, p, c, d]: chunk-pair t, intra-pair c; head rows (2t+c)*128+p
    head_v = head.rearrange("b (t c p) d -> b t p c d", c=2, p=P)
    # dep rows p*8 + 2t + c: s_d [128, 8] then flattens partition-major
    # into the natural j order for the row scatter
    dep_v = dep.rearrange("b (p t c) d -> b t p c d", t=NPAIR, c=2)
    # output pair view: row = t*256 + c*128 + p, flatten (p, c, j)
    out_v = out.rearrange("b (t c p) j -> b t p c j", c=2, p=P)

    with tile.TileContext(nc) as tc:
        with (
            tc.tile_pool(name="singles", bufs=1) as singles,
            tc.tile_pool(name="loads", bufs=2 * NPAIR) as loads,
            tc.tile_pool(name="svec", bufs=2) as svec,
            tc.tile_pool(name="scratch", bufs=3) as scratch,
            tc.tile_pool(name="rows", bufs=2) as rows,
            tc.tile_pool(name="bcast", bufs=2) as bcast,
            tc.tile_pool(name="outs", bufs=4) as outs,
            tc.tile_pool(name="psum", bufs=2, space="PSUM") as psum,
        ):
            wt = singles.tile([P, 2 * D], F32)
            nc.gpsimd.dma_start(out=wt, in_=w.to_broadcast([P, 2 * D]))
            bt = singles.tile([P, 1], F32)
            nc.gpsimd.dma_start(out=bt, in_=b.to_broadcast([P, 1]))
            ones = singles.tile([1, P], F32)
            nc.vector.memset(ones, 1.0)

            def eng(name):
                return {"V": nc.vector, "A": nc.scalar, "G": nc.gpsimd}[name]

            for bi in range(BPC):
                # ---- loads: dep first (it gates the broadcast row) ----
                dep_t = []
                for t in range(NPAIR):
                    dt_ = loads.tile([P, 2, D], F32, tag="dep")
                    nc.sync.dma_start(out=dt_, in_=dep_v[bi, t])
                    dep_t.append(dt_)
                head_t = []
                for t in range(NPAIR):
                    ht = loads.tile([P, 2, D], F32, tag="head")
                    nc.sync.dma_start(out=ht, in_=head_v[bi, t])
                    head_t.append(ht)

                # ---- s_d = dep . w_d ----
                sd = svec.tile([P, C], F32, tag="sd")
                for k in range(C):
                    src = dep_t[k // 2][:, k % 2, :]
                    prod = scratch.tile([P, D], F32, tag="prod")
                    eng(DEP_MULT_ENG[k]).tensor_mul(prod, src, wt[:, D : 2 * D])
                    trash = scratch.tile([P, D], F32, tag="trash")
                    nc.scalar.activation(
                        out=trash,
                        in_=prod,
                        func=mybir.ActivationFunctionType.Copy,
                        accum_out=sd[:, k : k + 1],
                    )
                # fold bias in before the scatter so every consumer gets it
                sd_b = svec.tile([P, C], F32, tag="sdb")
                nc.vector.tensor_scalar_add(sd_b, sd, bt)

                # s_d [128, 8] -> [1, 1024] row: row[0, p*8+k] = sd_b[p, k]
                sd_row = rows.tile([1, S], F32, tag="sdrow")
                nc.scalar.dma_start(out=sd_row, in_=sd_b)

                # broadcast the row to all partitions (PSUM), and one SBUF
                # copy so the DVE-side output adds run in 2x perf mode
                ps = psum.tile([P, S], F32, tag="ps")
                nc.tensor.matmul(
                    ps[:, 0:512], lhsT=ones, rhs=sd_row[:, 0:512], start=True, stop=True
                )
                nc.tensor.matmul(
                    ps[:, 512:1024],
                    lhsT=ones,
                    rhs=sd_row[:, 512:1024],
                    start=True,
                    stop=True,
                )
                sdb_sb = bcast.tile([P, S], F32, tag="sdbsb")
                nc.scalar.copy(out=sdb_sb, in_=ps)

                # ---- s_h chunks + output chunks ----
                sh = svec.tile([P, C], F32, tag="sh")
                for t in range(NPAIR):
                    ot = outs.tile([P, 2, S], F32, tag="ot")
                    for i in range(2):
                        c = 2 * t + i
                        src = head_t[t][:, i, :]
                        prod = scratch.tile([P, D], F32, tag="prod")
                        eng(HEAD_MULT_ENG[c]).tensor_mul(prod, src, wt[:, 0:D])
                        if HEAD_RED_ENG[c] == "A":
                            trash = scratch.tile([P, D], F32, tag="trash")
                            nc.scalar.activation(
                                out=trash,
                                in_=prod,
                                func=mybir.ActivationFunctionType.Copy,
                                accum_out=sh[:, c : c + 1],
                            )
                        else:
                            nc.vector.reduce_sum(
                                sh[:, c : c + 1], prod, axis=mybir.AxisListType.X
                            )
                        if OUT_ENG[c] == "A":
                            nc.scalar.add(
                                out=ot[:, i, :], in_=ps, add=sh[:, c : c + 1]
                            )
                        else:
                            nc.vector.tensor_scalar_add(
                                ot[:, i, :], sdb_sb, sh[:, c : c + 1]
                            )
                    nc.scalar.dma_start(out=out_v[bi, t], in_=ot)
    nc.compile()
    return nc


def kernel(head, dep, edge_W, edge_b, _trace=False):
    nc = build_program()
    in_maps = []
    for k in range(N_CORES):
        in_maps.append(
            {
                "head": np.ascontiguousarray(head[k * BPC : (k + 1) * BPC]),
                "dep": np.ascontiguousarray(dep[k * BPC : (k + 1) * BPC]),
                "edge_W": np.ascontiguousarray(edge_W),
                "edge_b": np.ascontiguousarray(edge_b),
            }
        )
    res = run_bass_kernel_spmd(nc, in_maps, core_ids=list(range(N_CORES)), trace=_trace)
    out = np.concatenate([r["out"] for r in res.results], axis=0)
    if _trace:
        return out, res
    return out


if __name__ == "__main__":
    rng = np.random.default_rng(0)
    head = rng.standard_normal((B, S, D), dtype=np.float32)
    dep = rng.standard_normal((B, S, D), dtype=np.float32)
    edge_W = rng.standard_normal((1, 2 * D), dtype=np.float32)
    edge_b = rng.standard_normal((1,), dtype=np.float32)
    out = kernel(head, dep, edge_W, edge_b)
    ref = (
        head @ edge_W[0, :D]
    )[:, :, None] + (dep @ edge_W[0, D:])[:, None, :] + edge_b[0]
    err = np.abs(out - ref).max() / np.abs(ref).max()
    print("max rel err:", err)
